# revision 52
# baseline (speedup 1.0000x reference)
"""Trainium2 Bass kernel for nn_AttentionHierarchy (BiGRU + attention pooling).

Time-sliced sharding (v2): wall time of the GRU recurrence is serial-steps x
per-step chain latency, so instead of data-parallel over batch we split TIME:

  core c (c=0..7): original-time slice t in [128c, 128c+128) for ALL 64 seqs,
  running TWO interleaved chains (fwd + bwd GRU, batch 64 each). Each chain
  starts K=32 steps early from h=0: the GRU forgets its initial state at
  ~0.5/step, so after 32 warmup steps the state matches the full scan to
  ~2e-6 (validated on the actual weight draw). Chains whose warmup would
  precede t=0 (core 0 fwd, core 7 bwd) get host-built "freezer" tokens:
  x* solves W_ih_z x* = 26 - b_z so the z-gate saturates to 1 and h stays
  exactly 0 through warmup.

  The bwd chain of core c covers reversed-time [128(7-c)-K, 128(8-c)), i.e.
  the SAME original-t range as its fwd slice, so attention (which mixes
  fwd/bwd halves at equal t) is core-local: no enc exchange at all. bwd enc
  is mirror-written so both tiles are t-ascending.

  Each core computes unnormalized attention partials for its slice
  (Num[h,b] = sum_t e^logit * enc, Den[b] = sum_t e^logit, masked); one
  small AllReduce(add) over all 8 cores combines them; out = Num/Den.

Per-step layouts match the v1 kernel: gates on partitions (6 chunks of 128),
batch on free dim; x pre-gates from a blocked 512-wide GEMM; h-recurrence as
12 weight-stationary matmuls (N=64) + 2 rank-1 bias matmuls per chain-step.
"""

import numpy as np
from contextlib import ExitStack

import concourse.bass as bass
import concourse.bacc as bacc
import concourse.mybir as mybir
from concourse import tile
from concourse.bass_utils import run_bass_kernel_spmd

F32 = mybir.dt.float32
AF = mybir.ActivationFunctionType
ALU = mybir.AluOpType
AX = mybir.AxisListType

B, T, D, H = 64, 1024, 300, 512
Hd = H // 2          # 256
H3 = 3 * Hd          # 768
NC = 8
ST = T // NC         # 128 slice steps per core
K = 32               # warmup steps
TS = ST + K          # 160 chain steps per direction
SB = 16              # steps per block
NBLK = TS // SB      # 10
DCH = 3              # d chunks (300 -> 384)
CCH = 2              # hidden chunks (256)
MCH = 6              # gate chunks (768)
NTOK = TS * B        # tokens per chain
PAD = 2 * SB * B     # prefetch overrun pad (2 blocks)

NW = 2 * DCH * H3 + 2 * CCH * H3 + 4 * H + 4
NF = 2 * CCH + 4
BIAS_ROW = D - 2 * 128  # row 44 of the c=2 w_ih chunk carries the biases


def build_program(bf16=True, debug_stage=3):
    XD = mybir.dt.bfloat16 if bf16 else F32
    nc = bacc.Bacc()

    xT = [nc.dram_tensor(nm, [DCH, 128, NTOK + PAD], XD, kind="ExternalInput")
          for nm in ("xTf", "xTb")]
    wblob = nc.dram_tensor("wblob", [128, NW], XD, kind="ExternalInput")
    fblob = nc.dram_tensor("fblob", [128, NF], F32, kind="ExternalInput")
    amask_d = nc.dram_tensor("amask_d", [1, B * ST], F32, kind="ExternalInput")
    out = nc.dram_tensor("out", [H, B], F32, kind="ExternalOutput")
    enc = nc.dram_tensor("enc", [2, CCH, 128, B, TS], XD)
    num_d = nc.dram_tensor("num_d", [128, 4 * B + B], F32)
    num_o = nc.dram_tensor("num_o", [128, 4 * B + B], F32)

    # wblob column offsets
    W_IH = [0, DCH * H3]                       # per dir
    W_HH = [2 * DCH * H3, 2 * DCH * H3 + CCH * H3]
    AW0 = 2 * DCH * H3 + 2 * CCH * H3
    CTX0 = AW0 + 4 * H

    with ExitStack() as ctx:
        tc = ctx.enter_context(tile.TileContext(nc))
        wpool = ctx.enter_context(tc.tile_pool(name="weights", bufs=1))

        wblob_sb = wpool.tile([128, NW], XD)
        fblob_sb = wpool.tile([128, NF], F32)
        zcol = wpool.tile([128, 1], F32)
        ones_sb = wpool.tile([1, 128], F32)
        ones1 = wpool.tile([1, B], XD)
        nc.sync.dma_start(wblob_sb[:], wblob[:])
        nc.sync.dma_start(fblob_sb[:], fblob[:])
        ones2 = wpool.tile([128, CCH, B], F32)
        nc.gpsimd.memset(zcol[:], 0.0)
        nc.gpsimd.memset(ones_sb[:], 1.0)
        nc.gpsimd.memset(ones1[:], 1.0)
        nc.gpsimd.memset(ones2[:], 1.0)

        w_ih = [[wblob_sb[:, W_IH[d] + c * H3: W_IH[d] + (c + 1) * H3]
                 for c in range(DCH)] for d in range(2)]
        w_hh = [[wblob_sb[:, W_HH[d] + c * H3: W_HH[d] + (c + 1) * H3]
                 for c in range(CCH)] for d in range(2)]
        aw_sb = {(d, c): wblob_sb[:, AW0 + (d * 2 + c) * H: AW0 + (d * 2 + c + 1) * H]
                 for d in range(2) for c in range(CCH)}
        ctxv_sb = wblob_sb[:, CTX0:CTX0 + 4]
        bhn_col = [[fblob_sb[:, d * CCH + c: d * CCH + c + 1]
                    for c in range(CCH)] for d in range(2)]
        attb_sb = fblob_sb[:, 2 * CCH: 2 * CCH + 4]

        # ---------------- recurrence (both dirs interleaved) --------------
        with (
            tc.tile_pool(name="fx", bufs=2) as xpool,
            tc.tile_pool(name="fgx", bufs=1) as fgx,
            tc.tile_pool(name="rec_col", bufs=2) as rcol,
            tc.tile_pool(name="rec_tmp", bufs=2) as rtmp,
            tc.tile_pool(name="rec_ps", bufs=1, space="PSUM") as rps,
            tc.tile_pool(name="gps", bufs=2, space="PSUM") as gps,
            tc.tile_pool(name="rec_h", bufs=1) as rh,
            tc.tile_pool(name="enc_pool", bufs=2) as encp,
        ):
            h_tile = [rh.tile([128, CCH, B], XD, name="h%d" % d) for d in range(2)]
            for d in range(2):
                nc.vector.memset(h_tile[d][:], 0.0)
            # gx layout: [128, (ht, m, t8, b)] so each GEMM tile copy is a
            # contiguous [128, 512] DMA from PSUM
            gxbuf = [[fgx.tile([128, 2, MCH, 8, B], F32, name="gx%d%d" % (d, a))
                      for a in range(2)] for d in range(2)]

            def gemm_xdma(d, blk_i):
                xs = []
                for c in range(DCH):
                    xt = xpool.tile([128, SB * B], XD, tag="xs%d%d" % (d, c),
                                    name="xs%d%d" % (d, c))
                    nc.sync.dma_start(
                        xt[:], xT[d][c, :, bass.ds(blk_i * (SB * B), SB * B)])
                    xs.append(xt)
                return xs

            def gemm_piece(d, xs, out_gx, ht, m):
                ps = gps.tile([128, 512], F32, tag="gps")
                for c in range(DCH):
                    nc.tensor.matmul(
                        ps[:],
                        w_ih[d][c][:, m * 128:(m + 1) * 128],
                        xs[c][:, ht * 512:(ht + 1) * 512],
                        start=(c == 0),
                        stop=(c == DCH - 1),
                    )
                ov = out_gx[:, ht, m].rearrange("p t b -> p (t b)")
                if (ht * MCH + m) % 2 == 0:
                    nc.scalar.activation(ov, ps[:], AF.Copy)
                else:
                    nc.vector.tensor_copy(ov, ps[:])

            def gemm_half(d, xs, out_gx, ht):
                for m in range(MCH):
                    gemm_piece(d, xs, out_gx, ht, m)

            def gemm_block(d, blk_i, out_gx):
                xs = gemm_xdma(d, blk_i)
                for ht in range(2):
                    gemm_half(d, xs, out_gx, ht)

            h_last = [None, None]

            def chain_step_a(d, s, gx_cur, st):
                """Phase 1: h-matmuls, rzs, sigma, t2/pre, zbar/zh."""
                if s == 0:
                    h_prev = h_tile[d][:, :, :]
                else:
                    h_prev = h_last[d][:, :, :]
                gp_rz = rps.tile([128, 4, B], F32, tag="gprz%d" % d)
                gp_n = rps.tile([128, CCH, B], F32, tag="gpn%d" % d)
                for m in range(MCH):
                    tgt = gp_rz[:, m] if m < 4 else gp_n[:, m - 4]
                    for c in range(CCH):
                        nc.tensor.matmul(
                            tgt,
                            w_hh[d][c][:, m * 128:(m + 1) * 128],
                            h_prev[:, c],
                            start=(c == 0),
                            stop=(c == CCH - 1),
                        )
                gxv = gx_cur[:, s // 8, :, s % 8]
                rzs = rtmp.tile([128, 4, B], F32, tag="rzs%d" % d)
                nc.vector.tensor_add(rzs[:], gp_rz[:], gxv[:, :4])
                rz = rtmp.tile([128, 4, B], F32, tag="rz%d" % d)
                nc.scalar.activation(rz[:], rzs[:], AF.Sigmoid, bias=zcol[:])
                t2 = rtmp.tile([128, CCH, B], F32, tag="t2%d" % d)
                for c in range(CCH):
                    nc.vector.scalar_tensor_tensor(
                        t2[:, c], gp_n[:, c], bhn_col[d][c], rz[:, c],
                        ALU.add, ALU.mult)
                pre = rtmp.tile([128, CCH, B], F32, tag="pre%d" % d)
                nc.vector.tensor_add(pre[:], t2[:], gxv[:, 4:])
                zbar = rtmp.tile([128, CCH, B], F32, tag="zb%d" % d)
                nc.vector.scalar_tensor_tensor(
                    zbar[:], rz[:, CCH:], -1.0, ones2[:], ALU.mult, ALU.add)
                zh = rtmp.tile([128, CCH, B], F32, tag="zh%d" % d)
                nc.gpsimd.tensor_mul(zh[:], rz[:, CCH:], h_prev)
                st[d] = (pre, zbar, zh)

            def chain_step_b(d, s, cols, enc_t, st):
                """Phase 2: tanh, update, state copies."""
                pre, zbar, zh = st[d]
                n = rtmp.tile([128, CCH, B], F32, tag="n%d" % d)
                nc.scalar.activation(n[:], pre[:], AF.Tanh, bias=zcol[:])
                zbn = rtmp.tile([128, CCH, B], F32, tag="zn%d" % d)
                nc.vector.tensor_mul(zbn[:], zbar[:], n[:])
                cur = rtmp.tile([128, CCH, B], XD, tag="hc%d" % d,
                                name="hc%d" % d)
                nc.vector.tensor_add(cur[:], zbn[:], zh[:])
                h_last[d] = cur
                cslot = cols[s // 8][:, :, :, s % 8]
                if s % 2 == 0:
                    nc.scalar.activation(cslot, cur[:], AF.Copy)
                else:
                    nc.gpsimd.tensor_copy(cslot, cur[:])
                if s == SB - 1:
                    nc.vector.tensor_copy(h_tile[d][:], cur[:])
                if s % 8 == 7:
                    sub = s // 8
                    if d == 0:
                        dst = enc_t[:, :, :, bass.ds(sub * 8, 8)]
                    else:
                        dst = enc_t[:, :, :, bass.ds(8 - 8 * sub, 8)][:, :, :, ::-1]
                    nc.scalar.activation(dst, cols[sub][:], AF.Copy)

            def block(blk_i, ab, pf_ab):
                """One 16-step block for both dirs. blk_i: block index expr;
                ab: which gx buffer each dir consumes; pf_ab: prefetch target."""
                cols = [[rcol.tile([128, CCH, B, 8], XD,
                                   tag="col%d%d" % (d, kk),
                                   name="col%d%d" % (d, kk))
                         for kk in range(2)] for d in range(2)]
                enc_t = [encp.tile([128, CCH, B, SB], XD, tag="enc%d" % d,
                                   name="enct%d" % d) for d in range(2)]
                xs_pf = [None, None]
                st = [None, None]
                for s in range(SB):
                    for d in range(2):
                        chain_step_a(d, s, gxbuf[d][ab][:], st)
                    for d in range(2):
                        chain_step_b(d, s, cols[d], enc_t[d], st)
                    if s == 0:
                        xs_pf[0] = gemm_xdma(0, blk_i + 1)
                        xs_pf[1] = gemm_xdma(1, blk_i + 1)
                    elif s == 1:
                        gemm_half(0, xs_pf[0], gxbuf[0][pf_ab], 0)
                    elif s == 5:
                        gemm_half(0, xs_pf[0], gxbuf[0][pf_ab], 1)
                    elif s == 8:
                        gemm_half(1, xs_pf[1], gxbuf[1][pf_ab], 0)
                    elif s == 12:
                        gemm_half(1, xs_pf[1], gxbuf[1][pf_ab], 1)
                for d in range(2):
                    for c in range(CCH):
                        if d == 0:
                            dst = enc[0, c][:, :, bass.ds(blk_i * SB, SB)]
                        else:
                            dst = enc[1, c][:, :, bass.ds((NBLK - 1) * SB - blk_i * SB, SB)]
                        nc.sync.dma_start(dst, enc_t[d][:, c])

            gemm_block(0, 0, gxbuf[0][0])
            gemm_block(1, 0, gxbuf[1][0])
            with tc.For_i(0, NBLK // 2, 1) as i:
                block(i * 2, 0, 1)
                block(i * 2 + 1, 1, 0)

        if debug_stage < 2:
            ov = out[:]
            for q in range(4):
                d, c = q // 2, q % 2
                with nc.allow_non_contiguous_dma(reason="debug"):
                    nc.gpsimd.dma_start(
                        ov[q * 128:(q + 1) * 128, :],
                        enc[d, c, :, :, K if d == 0 else 0],
                    )
        else:
            # ---------------- attention partials --------------------------
            with (
                tc.tile_pool(name="att_u", bufs=2) as upool,
                tc.tile_pool(name="att_s", bufs=2) as spool,
                tc.tile_pool(name="att_keep", bufs=1) as kpool,
            ):
                num_sb = kpool.tile([128, 4 * B + B], F32)
                nc.vector.memset(num_sb[:], 0.0)
                P = num_sb[:, :4 * B].rearrange("p (q b) -> p q b", b=B)
                dens = num_sb[0:1, 4 * B:]
                am_flat = kpool.tile([1, B * ST], F32)
                nc.sync.dma_start(am_flat[:], amask_d[:])
                # enc preload DMAs issued before the engine barrier so they
                # overlap the recurrence drain (DRAM deps order them after
                # the in-loop enc writes)
                bige = {}
                for d in range(2):
                    for c in range(CCH):
                        e = kpool.tile([128, B, ST], XD, name="be%d%d" % (d, c))
                        nc.sync.dma_start(
                            e[:], enc[d, c, :, :,
                                      bass.ds(K if d == 0 else 0, ST)])
                        bige[(d, c)] = e
                tc.strict_bb_all_engine_barrier()
                aps = ctx.enter_context(
                    tc.tile_pool(name="att_ps", bufs=1, space="PSUM"))
                lps = ctx.enter_context(
                    tc.tile_pool(name="att_lg", bufs=2, space="PSUM"))
                bpool = ctx.enter_context(
                    tc.tile_pool(name="att_bc", bufs=2, space="PSUM"))
                for j in range(B // 4):
                    b0 = 4 * j
                    encs = {k: v[:, b0:b0 + 4, :] for k, v in bige.items()}
                    ups = [aps.tile([128, 512], F32, tag="ups%d" % m,
                                    name="ups%d" % m) for m in range(4)]
                    ki = 0
                    for d in range(2):
                        for c in range(CCH):
                            rhs = encs[(d, c)][:].rearrange("p b t -> p (b t)")
                            for m in range(4):
                                nc.tensor.matmul(
                                    ups[m][:],
                                    aw_sb[(d, c)][:, m * 128:(m + 1) * 128],
                                    rhs,
                                    start=(ki == 0), stop=(ki == 3),
                                )
                            ki += 1
                    u_sb = upool.tile([128, 4, 512], XD, tag="usb")
                    for m in range(4):
                        nc.scalar.activation(
                            u_sb[:, m], ups[m][:], AF.Tanh,
                            bias=attb_sb[:, m:m + 1])
                    lg = lps.tile([1, 512], F32, tag="lg")
                    for m in range(4):
                        nc.tensor.matmul(
                            lg[:], ctxv_sb[:, m:m + 1], u_sb[:, m],
                            start=(m == 0), stop=(m == 3))
                    e_row = spool.tile([1, 512], F32, tag="erow")
                    nc.scalar.activation(e_row[:], lg[:], AF.Exp,
                                         bias=zcol[0:1, :])
                    if debug_stage == 25:
                        nc.sync.dma_start(out[b0 * 8:b0 * 8 + 8, :],
                                          e_row[:].rearrange("o (x b) -> (o x) b", b=B))
                        continue
                    em = spool.tile([1, 4 * ST], F32, tag="em")
                    nc.vector.tensor_mul(
                        em[:], e_row[:],
                        am_flat[0:1, b0 * ST:(b0 + 4) * ST])
                    nc.vector.tensor_reduce(
                        dens[0:1, b0:b0 + 4],
                        em[:].rearrange("o (b t) -> o b t", t=ST),
                        AX.X, ALU.add)
                    bc = bpool.tile([128, 4 * ST], F32, tag="bc")
                    nc.tensor.matmul(bc[:], ones_sb[:], em[:],
                                     start=True, stop=True)
                    bc_sb = spool.tile([128, 4 * ST], F32, tag="bcs")
                    nc.scalar.activation(bc_sb[:], bc[:], AF.Copy)
                    for d in range(2):
                        for c in range(CCH):
                            q = d * 2 + c
                            ef = encs[(d, c)][:].rearrange("p b t -> p (b t)")
                            scr = spool.tile([128, 4 * ST], F32,
                                             tag="scr%d" % (q % 2))
                            nc.vector.tensor_mul(scr[:], ef, bc_sb[:])
                            nc.vector.tensor_reduce(
                                P[:, q, b0:b0 + 4],
                                scr[:].rearrange("p (b t) -> p b t", t=ST),
                                AX.X, ALU.add)
                if debug_stage != 25:
                    nsum = kpool.tile([128, 4 * B + B], F32, name="nsum")
                    if debug_stage == 26:
                        nc.vector.tensor_copy(nsum[:], num_sb[:])
                    else:
                        nc.sync.dma_start(num_d[:], num_sb[:])
                        nc.gpsimd.collective_compute(
                            "AllReduce", ALU.add,
                            replica_groups=[list(range(NC))],
                            ins=[num_d[:]], outs=[num_o[:]],
                        )
                        nc.sync.dma_start(nsum[:], num_o[:])
                    rden = spool.tile([1, B], F32, tag="rden")
                    nc.vector.reciprocal(rden[:], nsum[0:1, 4 * B:])
                    rbc = bpool.tile([128, B], F32, tag="bc")
                    nc.tensor.matmul(rbc[:], ones_sb[:], rden[:],
                                     start=True, stop=True)
                    Ps = nsum[:, :4 * B].rearrange("p (q b) -> p q b", b=B)
                    for q in range(4):
                        oq = spool.tile([128, B], F32, tag="oq%d" % (q % 2))
                        nc.vector.tensor_mul(oq[:], Ps[:, q], rbc[:])
                        nc.sync.dma_start(out[q * 128:(q + 1) * 128, :], oq[:])

    nc.finalize()
    return nc


# ======================= host side =======================================

def _freezer_x(w_ih, b_ih, b_hh):
    """x* with W_z x* + b_z ~= 26: z-gate saturates, h frozen at 0."""
    Wz = np.asarray(w_ih, np.float64)[Hd:2 * Hd]
    tgt = 26.0 - np.asarray(b_ih, np.float64)[Hd:2 * Hd] \
        - np.asarray(b_hh, np.float64)[Hd:2 * Hd]
    xs = Wz.T @ np.linalg.solve(Wz @ Wz.T, tgt)
    return xs.astype(np.float32)


def _fmt_x(xs, xd):
    """[B, TS, D] -> [DCH, 128, TS*B + PAD] in (t-major, b) order.
    Pad row D is the constant-1 bias row."""
    nb, Tn = xs.shape[0], xs.shape[1]
    xp = np.zeros((nb, Tn, DCH * 128), np.float32)
    xp[:, :, :D] = xs
    xp[:, :, D] = 1.0
    xt = xp.reshape(nb, Tn, DCH, 128).transpose(2, 3, 1, 0)
    flat = np.ascontiguousarray(xt).reshape(DCH, 128, Tn * nb)
    outa = np.zeros((DCH, 128, Tn * nb + PAD), np.float32)
    outa[:, :, :Tn * nb] = flat
    return outa.astype(xd)


def _prep_inputs(x, lengths, w_ih_f, w_hh_f, b_ih_f, b_hh_f,
                 w_ih_b, w_hh_b, b_ih_b, b_hh_b, att_w, att_b, ctx_v,
                 bf16=True):
    import ml_dtypes
    xd = ml_dtypes.bfloat16 if bf16 else np.float32
    f32 = np.float32
    x = np.asarray(x, f32)
    lengths = np.asarray(lengths).astype(np.int64)

    mask = (np.arange(T)[None, :] < lengths[:, None])
    xm = x * mask[:, :, None].astype(f32)
    xmr = np.ascontiguousarray(xm[:, ::-1])

    xf_f = _freezer_x(w_ih_f, b_ih_f, b_hh_f)
    xf_b = _freezer_x(w_ih_b, b_ih_b, b_hh_b)

    wb = np.zeros((128, NW), f32)
    for d, (wih, bi_, bh_) in enumerate(
            ((w_ih_f, b_ih_f, b_hh_f), (w_ih_b, b_ih_b, b_hh_b))):
        wt = np.asarray(wih, f32).T
        for c in range(DCH):
            d0, d1 = c * 128, min((c + 1) * 128, D)
            if d0 < D:
                wb[: d1 - d0, d * DCH * H3 + c * H3:
                   d * DCH * H3 + (c + 1) * H3] = wt[d0:d1]
        # bias rides the constant-1 pad row of x (chunk 2, row 44):
        # rz gates get b_ih+b_hh, n gates get b_ih only
        bias = np.asarray(bi_, f32).copy()
        bias[:2 * Hd] += np.asarray(bh_, f32)[:2 * Hd]
        wb[BIAS_ROW, d * DCH * H3 + 2 * H3: d * DCH * H3 + 3 * H3] = bias
    W_HH0 = 2 * DCH * H3
    for d, whh in enumerate((w_hh_f, w_hh_b)):
        ht = np.asarray(whh, f32).T
        for c in range(CCH):
            wb[:, W_HH0 + d * CCH * H3 + c * H3:
               W_HH0 + d * CCH * H3 + (c + 1) * H3] = ht[c * 128:(c + 1) * 128]
    AW0 = 2 * DCH * H3 + 2 * CCH * H3
    awt = np.asarray(att_w, f32).T
    for d in range(2):
        for c in range(CCH):
            q = d * 2 + c
            wb[:, AW0 + q * H: AW0 + (q + 1) * H] = awt[
                d * Hd + c * 128: d * Hd + (c + 1) * 128]
    CTX0 = AW0 + 4 * H
    wb[:, CTX0:CTX0 + 4] = np.asarray(ctx_v, f32).reshape(4, 128).T
    wb = wb.astype(xd)

    fb = np.zeros((128, NF), f32)
    for d, bh_ in enumerate((b_hh_f, b_hh_b)):
        bh = np.asarray(bh_, f32)
        for c in range(CCH):
            fb[:, d * CCH + c] = bh[2 * Hd + c * 128: 2 * Hd + (c + 1) * 128]
    fb[:, 2 * CCH:] = np.asarray(att_b, f32).reshape(4, 128).T

    tidx = np.arange(T)
    in_maps = []
    for c in range(NC):
        # fwd chain tokens: global tau = 128c - K + s
        t0 = c * ST - K
        xs_f = np.zeros((B, TS, D), f32)
        lo = max(0, -t0)
        xs_f[:, lo:] = xm[:, t0 + lo: t0 + TS]
        if lo:
            xs_f[:, :lo] = xf_f[None, None, :]
        # bwd chain: reversed-time tau_r = 128(7-c) - K + s
        t0r = (NC - 1 - c) * ST - K
        xs_b = np.zeros((B, TS, D), f32)
        lor = max(0, -t0r)
        xs_b[:, lor:] = xmr[:, t0r + lor: t0r + TS]
        if lor:
            xs_b[:, :lor] = xf_b[None, None, :]
        am = ((tidx[None, c * ST:(c + 1) * ST] < lengths[:, None])
              ).astype(f32)
        in_maps.append({
            "xTf": _fmt_x(xs_f, xd),
            "xTb": _fmt_x(xs_b, xd),
            "wblob": wb, "fblob": fb,
            "amask_d": np.ascontiguousarray(am.reshape(1, -1)),
        })
    return in_maps


_CACHED = {}

USE_BF16 = True


def kernel(**inputs):
    if "prog" not in _CACHED:
        _CACHED["prog"] = build_program(bf16=USE_BF16)
    nc = _CACHED["prog"]
    in_maps = _prep_inputs(**inputs, bf16=USE_BF16)
    res = run_bass_kernel_spmd(nc, in_maps, list(range(NC)))
    return np.ascontiguousarray(np.asarray(res.results[0]["out"], np.float32).T)


# revision 53
# speedup vs baseline: 1.1455x; 1.1455x over previous
"""Trainium2 Bass kernel for nn_AttentionHierarchy (BiGRU + attention pooling).

Time-sliced sharding (v2): wall time of the GRU recurrence is serial-steps x
per-step chain latency, so instead of data-parallel over batch we split TIME:

  core c (c=0..7): original-time slice t in [128c, 128c+128) for ALL 64 seqs,
  running TWO interleaved chains (fwd + bwd GRU, batch 64 each). Each chain
  starts K=32 steps early from h=0: the GRU forgets its initial state at
  ~0.5/step, so after 32 warmup steps the state matches the full scan to
  ~2e-6 (validated on the actual weight draw). Chains whose warmup would
  precede t=0 (core 0 fwd, core 7 bwd) get host-built "freezer" tokens:
  x* solves W_ih_z x* = 26 - b_z so the z-gate saturates to 1 and h stays
  exactly 0 through warmup.

  The bwd chain of core c covers reversed-time [128(7-c)-K, 128(8-c)), i.e.
  the SAME original-t range as its fwd slice, so attention (which mixes
  fwd/bwd halves at equal t) is core-local: no enc exchange at all. bwd enc
  is mirror-written so both tiles are t-ascending.

  Each core computes unnormalized attention partials for its slice
  (Num[h,b] = sum_t e^logit * enc, Den[b] = sum_t e^logit, masked); one
  small AllReduce(add) over all 8 cores combines them; out = Num/Den.

Per-step layouts match the v1 kernel: gates on partitions (6 chunks of 128),
batch on free dim; x pre-gates from a blocked 512-wide GEMM; h-recurrence as
12 weight-stationary matmuls (N=64) + 2 rank-1 bias matmuls per chain-step.
"""

import numpy as np
from contextlib import ExitStack

import concourse.bass as bass
import concourse.bacc as bacc
import concourse.mybir as mybir
from concourse import tile
from concourse.bass_utils import run_bass_kernel_spmd

F32 = mybir.dt.float32
AF = mybir.ActivationFunctionType
ALU = mybir.AluOpType
AX = mybir.AxisListType

B, T, D, H = 64, 1024, 300, 512
Hd = H // 2          # 256
H3 = 3 * Hd          # 768
NC = 8
ST = T // NC         # 128 slice steps per core
K = 32               # warmup steps
TS = ST + K          # 160 chain steps per direction
SB = 16              # steps per block
NBLK = TS // SB      # 10
DCH = 3              # d chunks (300 -> 384)
CCH = 2              # hidden chunks (256)
MCH = 6              # gate chunks (768)
NTOK = TS * B        # tokens per chain
PAD = 2 * SB * B     # prefetch overrun pad (2 blocks)

NW = 2 * DCH * H3 + 2 * CCH * H3 + 4 * H + 4
NF = 2 * CCH + 4
BIAS_ROW = D - 2 * 128  # row 44 of the c=2 w_ih chunk carries the biases


def build_program(bf16=True, debug_stage=3):
    XD = mybir.dt.bfloat16 if bf16 else F32
    nc = bacc.Bacc()

    xT = [nc.dram_tensor(nm, [DCH, 128, NTOK + PAD], XD, kind="ExternalInput")
          for nm in ("xTf", "xTb")]
    wblob = nc.dram_tensor("wblob", [128, NW], XD, kind="ExternalInput")
    fblob = nc.dram_tensor("fblob", [128, NF], F32, kind="ExternalInput")
    amask_d = nc.dram_tensor("amask_d", [1, B * ST], F32, kind="ExternalInput")
    out = nc.dram_tensor("out", [H, B], F32, kind="ExternalOutput")
    enc = nc.dram_tensor("enc", [2, CCH, 128, B, TS], XD)
    num_d = nc.dram_tensor("num_d", [128, 4 * B + B], F32)
    num_o = nc.dram_tensor("num_o", [128, 4 * B + B], F32)

    # wblob column offsets
    W_IH = [0, DCH * H3]                       # per dir
    W_HH = [2 * DCH * H3, 2 * DCH * H3 + CCH * H3]
    AW0 = 2 * DCH * H3 + 2 * CCH * H3
    CTX0 = AW0 + 4 * H

    with ExitStack() as ctx:
        tc = ctx.enter_context(tile.TileContext(nc))
        wpool = ctx.enter_context(tc.tile_pool(name="weights", bufs=1))

        wblob_sb = wpool.tile([128, NW], XD)
        fblob_sb = wpool.tile([128, NF], F32)
        zcol = wpool.tile([128, 1], F32)
        ones_sb = wpool.tile([1, 128], F32)
        ones1 = wpool.tile([1, B], XD)
        nc.sync.dma_start(wblob_sb[:], wblob[:])
        nc.sync.dma_start(fblob_sb[:], fblob[:])
        ones2 = wpool.tile([128, CCH, B], F32)
        nc.gpsimd.memset(zcol[:], 0.0)
        nc.gpsimd.memset(ones_sb[:], 1.0)
        nc.gpsimd.memset(ones1[:], 1.0)
        nc.gpsimd.memset(ones2[:], 1.0)

        w_ih = [[wblob_sb[:, W_IH[d] + c * H3: W_IH[d] + (c + 1) * H3]
                 for c in range(DCH)] for d in range(2)]
        w_hh = [[wblob_sb[:, W_HH[d] + c * H3: W_HH[d] + (c + 1) * H3]
                 for c in range(CCH)] for d in range(2)]
        aw_sb = {(d, c): wblob_sb[:, AW0 + (d * 2 + c) * H: AW0 + (d * 2 + c + 1) * H]
                 for d in range(2) for c in range(CCH)}
        ctxv_sb = wblob_sb[:, CTX0:CTX0 + 4]
        bhn_col = [[fblob_sb[:, d * CCH + c: d * CCH + c + 1]
                    for c in range(CCH)] for d in range(2)]
        attb_sb = fblob_sb[:, 2 * CCH: 2 * CCH + 4]

        # ---------------- recurrence (both dirs interleaved) --------------
        with (
            tc.tile_pool(name="fx", bufs=2) as xpool,
            tc.tile_pool(name="fgx", bufs=1) as fgx,
            tc.tile_pool(name="rec_col", bufs=2) as rcol,
            tc.tile_pool(name="rec_tmp", bufs=2) as rtmp,
            tc.tile_pool(name="rec_ps", bufs=1, space="PSUM") as rps,
            tc.tile_pool(name="gps", bufs=2, space="PSUM") as gps,
            tc.tile_pool(name="rec_h", bufs=1) as rh,
            tc.tile_pool(name="enc_pool", bufs=2) as encp,
        ):
            h_tile = [rh.tile([128, CCH, B], XD, name="h%d" % d) for d in range(2)]
            for d in range(2):
                nc.vector.memset(h_tile[d][:], 0.0)
            # gx layout: [128, (ht, m, t8, b)] so each GEMM tile copy is a
            # contiguous [128, 512] DMA from PSUM
            gxbuf = [[fgx.tile([128, 2, MCH, 8, B], F32, name="gx%d%d" % (d, a))
                      for a in range(2)] for d in range(2)]

            def gemm_xdma(d, blk_i):
                xs = []
                for c in range(DCH):
                    xt = xpool.tile([128, SB * B], XD, tag="xs%d%d" % (d, c),
                                    name="xs%d%d" % (d, c))
                    nc.sync.dma_start(
                        xt[:], xT[d][c, :, bass.ds(blk_i * (SB * B), SB * B)])
                    xs.append(xt)
                return xs

            def gemm_piece(d, xs, out_gx, ht, m):
                ps = gps.tile([128, 512], F32, tag="gps")
                for c in range(DCH):
                    nc.tensor.matmul(
                        ps[:],
                        w_ih[d][c][:, m * 128:(m + 1) * 128],
                        xs[c][:, ht * 512:(ht + 1) * 512],
                        start=(c == 0),
                        stop=(c == DCH - 1),
                    )
                ov = out_gx[:, ht, m].rearrange("p t b -> p (t b)")
                if (ht * MCH + m) % 2 == 0:
                    nc.scalar.activation(ov, ps[:], AF.Copy)
                else:
                    nc.vector.tensor_copy(ov, ps[:])

            def gemm_half(d, xs, out_gx, ht):
                for m in range(MCH):
                    gemm_piece(d, xs, out_gx, ht, m)

            def gemm_block(d, blk_i, out_gx):
                xs = gemm_xdma(d, blk_i)
                for ht in range(2):
                    gemm_half(d, xs, out_gx, ht)

            h_last = [None, None]

            def chain_step_a(d, s, gx_cur, st):
                """Phase 1: h-matmuls, rzs, sigma, t2/pre, zbar/zh."""
                if s == 0:
                    h_prev = h_tile[d][:, :, :]
                else:
                    h_prev = h_last[d][:, :, :]
                gp_rz = rps.tile([128, 4, B], F32, tag="gprz%d" % d)
                gp_n = rps.tile([128, CCH, B], F32, tag="gpn%d" % d)
                for m in range(MCH):
                    tgt = gp_rz[:, m] if m < 4 else gp_n[:, m - 4]
                    for c in range(CCH):
                        nc.tensor.matmul(
                            tgt,
                            w_hh[d][c][:, m * 128:(m + 1) * 128],
                            h_prev[:, c],
                            start=(c == 0),
                            stop=(c == CCH - 1),
                        )
                gxv = gx_cur[:, s // 8, :, s % 8]
                rzs = rtmp.tile([128, 4, B], F32, tag="rzs%d" % d)
                nc.vector.tensor_add(rzs[:], gp_rz[:], gxv[:, :4])
                rz = rtmp.tile([128, 4, B], F32, tag="rz%d" % d)
                nc.scalar.activation(rz[:], rzs[:], AF.Sigmoid, bias=zcol[:])
                t2 = rtmp.tile([128, CCH, B], F32, tag="t2%d" % d)
                for c in range(CCH):
                    nc.vector.scalar_tensor_tensor(
                        t2[:, c], gp_n[:, c], bhn_col[d][c], rz[:, c],
                        ALU.add, ALU.mult)
                pre = rtmp.tile([128, CCH, B], F32, tag="pre%d" % d)
                nc.vector.tensor_add(pre[:], t2[:], gxv[:, 4:])
                zbar = rtmp.tile([128, CCH, B], F32, tag="zb%d" % d)
                nc.vector.scalar_tensor_tensor(
                    zbar[:], rz[:, CCH:], -1.0, ones2[:], ALU.mult, ALU.add)
                zh = rtmp.tile([128, CCH, B], F32, tag="zh%d" % d)
                nc.gpsimd.tensor_mul(zh[:], rz[:, CCH:], h_prev)
                st[d] = (pre, zbar, zh)

            def chain_step_b(d, s, cols, enc_t, st):
                """Phase 2: tanh, update, state copies."""
                pre, zbar, zh = st[d]
                n = rtmp.tile([128, CCH, B], F32, tag="n%d" % d)
                nc.scalar.activation(n[:], pre[:], AF.Tanh, bias=zcol[:])
                zbn = rtmp.tile([128, CCH, B], F32, tag="zn%d" % d)
                nc.vector.tensor_mul(zbn[:], zbar[:], n[:])
                cur = rtmp.tile([128, CCH, B], XD, tag="hc%d" % d,
                                name="hc%d" % d)
                nc.vector.tensor_add(cur[:], zbn[:], zh[:])
                h_last[d] = cur
                cslot = cols[s // 8][:, :, :, s % 8]
                if s % 2 == 0:
                    nc.scalar.activation(cslot, cur[:], AF.Copy)
                else:
                    nc.gpsimd.tensor_copy(cslot, cur[:])
                if s == SB - 1:
                    nc.vector.tensor_copy(h_tile[d][:], cur[:])
                if s % 8 == 7:
                    sub = s // 8
                    if d == 0:
                        dst = enc_t[:, :, :, bass.ds(sub * 8, 8)]
                    else:
                        dst = enc_t[:, :, :, bass.ds(8 - 8 * sub, 8)][:, :, :, ::-1]
                    nc.scalar.activation(dst, cols[sub][:], AF.Copy)

            def block(blk_i, ab, pf_ab):
                """One 16-step block for both dirs. blk_i: block index expr;
                ab: which gx buffer each dir consumes; pf_ab: prefetch target."""
                cols = [[rcol.tile([128, CCH, B, 8], XD,
                                   tag="col%d%d" % (d, kk),
                                   name="col%d%d" % (d, kk))
                         for kk in range(2)] for d in range(2)]
                enc_t = [encp.tile([128, CCH, B, SB], XD, tag="enc%d" % d,
                                   name="enct%d" % d) for d in range(2)]
                xs_pf = [None, None]
                st = [None, None]
                for s in range(SB):
                    for d in range(2):
                        chain_step_a(d, s, gxbuf[d][ab][:], st)
                        chain_step_b(d, s, cols[d], enc_t[d], st)
                    if s == 0:
                        xs_pf[0] = gemm_xdma(0, blk_i + 1)
                        xs_pf[1] = gemm_xdma(1, blk_i + 1)
                    elif s == 1:
                        gemm_half(0, xs_pf[0], gxbuf[0][pf_ab], 0)
                    elif s == 5:
                        gemm_half(0, xs_pf[0], gxbuf[0][pf_ab], 1)
                    elif s == 8:
                        gemm_half(1, xs_pf[1], gxbuf[1][pf_ab], 0)
                    elif s == 12:
                        gemm_half(1, xs_pf[1], gxbuf[1][pf_ab], 1)
                for d in range(2):
                    for c in range(CCH):
                        if d == 0:
                            dst = enc[0, c][:, :, bass.ds(blk_i * SB, SB)]
                        else:
                            dst = enc[1, c][:, :, bass.ds((NBLK - 1) * SB - blk_i * SB, SB)]
                        nc.sync.dma_start(dst, enc_t[d][:, c])

            gemm_block(0, 0, gxbuf[0][0])
            gemm_block(1, 0, gxbuf[1][0])
            with tc.For_i(0, NBLK // 2, 1) as i:
                block(i * 2, 0, 1)
                block(i * 2 + 1, 1, 0)

        if debug_stage < 2:
            ov = out[:]
            for q in range(4):
                d, c = q // 2, q % 2
                with nc.allow_non_contiguous_dma(reason="debug"):
                    nc.gpsimd.dma_start(
                        ov[q * 128:(q + 1) * 128, :],
                        enc[d, c, :, :, K if d == 0 else 0],
                    )
        else:
            # ---------------- attention partials --------------------------
            with (
                tc.tile_pool(name="att_u", bufs=2) as upool,
                tc.tile_pool(name="att_s", bufs=2) as spool,
                tc.tile_pool(name="att_keep", bufs=1) as kpool,
            ):
                num_sb = kpool.tile([128, 4 * B + B], F32)
                nc.vector.memset(num_sb[:], 0.0)
                P = num_sb[:, :4 * B].rearrange("p (q b) -> p q b", b=B)
                dens = num_sb[0:1, 4 * B:]
                am_flat = kpool.tile([1, B * ST], F32)
                nc.sync.dma_start(am_flat[:], amask_d[:])
                # enc preload DMAs issued before the engine barrier so they
                # overlap the recurrence drain (DRAM deps order them after
                # the in-loop enc writes)
                bige = {}
                for d in range(2):
                    for c in range(CCH):
                        e = kpool.tile([128, B, ST], XD, name="be%d%d" % (d, c))
                        nc.sync.dma_start(
                            e[:], enc[d, c, :, :,
                                      bass.ds(K if d == 0 else 0, ST)])
                        bige[(d, c)] = e
                tc.strict_bb_all_engine_barrier()
                aps = ctx.enter_context(
                    tc.tile_pool(name="att_ps", bufs=1, space="PSUM"))
                lps = ctx.enter_context(
                    tc.tile_pool(name="att_lg", bufs=2, space="PSUM"))
                bpool = ctx.enter_context(
                    tc.tile_pool(name="att_bc", bufs=2, space="PSUM"))
                for j in range(B // 4):
                    b0 = 4 * j
                    encs = {k: v[:, b0:b0 + 4, :] for k, v in bige.items()}
                    ups = [aps.tile([128, 512], F32, tag="ups%d" % m,
                                    name="ups%d" % m) for m in range(4)]
                    ki = 0
                    for d in range(2):
                        for c in range(CCH):
                            rhs = encs[(d, c)][:].rearrange("p b t -> p (b t)")
                            for m in range(4):
                                nc.tensor.matmul(
                                    ups[m][:],
                                    aw_sb[(d, c)][:, m * 128:(m + 1) * 128],
                                    rhs,
                                    start=(ki == 0), stop=(ki == 3),
                                )
                            ki += 1
                    u_sb = upool.tile([128, 4, 512], XD, tag="usb")
                    for m in range(4):
                        nc.scalar.activation(
                            u_sb[:, m], ups[m][:], AF.Tanh,
                            bias=attb_sb[:, m:m + 1])
                    lg = lps.tile([1, 512], F32, tag="lg")
                    for m in range(4):
                        nc.tensor.matmul(
                            lg[:], ctxv_sb[:, m:m + 1], u_sb[:, m],
                            start=(m == 0), stop=(m == 3))
                    e_row = spool.tile([1, 512], F32, tag="erow")
                    nc.scalar.activation(e_row[:], lg[:], AF.Exp,
                                         bias=zcol[0:1, :])
                    if debug_stage == 25:
                        nc.sync.dma_start(out[b0 * 8:b0 * 8 + 8, :],
                                          e_row[:].rearrange("o (x b) -> (o x) b", b=B))
                        continue
                    em = spool.tile([1, 4 * ST], F32, tag="em")
                    nc.vector.tensor_mul(
                        em[:], e_row[:],
                        am_flat[0:1, b0 * ST:(b0 + 4) * ST])
                    nc.vector.tensor_reduce(
                        dens[0:1, b0:b0 + 4],
                        em[:].rearrange("o (b t) -> o b t", t=ST),
                        AX.X, ALU.add)
                    bc = bpool.tile([128, 4 * ST], F32, tag="bc")
                    nc.tensor.matmul(bc[:], ones_sb[:], em[:],
                                     start=True, stop=True)
                    bc_sb = spool.tile([128, 4 * ST], F32, tag="bcs")
                    nc.scalar.activation(bc_sb[:], bc[:], AF.Copy)
                    for d in range(2):
                        for c in range(CCH):
                            q = d * 2 + c
                            ef = encs[(d, c)][:].rearrange("p b t -> p (b t)")
                            scr = spool.tile([128, 4 * ST], F32,
                                             tag="scr%d" % (q % 2))
                            nc.vector.tensor_mul(scr[:], ef, bc_sb[:])
                            nc.vector.tensor_reduce(
                                P[:, q, b0:b0 + 4],
                                scr[:].rearrange("p (b t) -> p b t", t=ST),
                                AX.X, ALU.add)
                if debug_stage != 25:
                    nsum = kpool.tile([128, 4 * B + B], F32, name="nsum")
                    if debug_stage == 26:
                        nc.vector.tensor_copy(nsum[:], num_sb[:])
                    else:
                        nc.sync.dma_start(num_d[:], num_sb[:])
                        nc.gpsimd.collective_compute(
                            "AllReduce", ALU.add,
                            replica_groups=[list(range(NC))],
                            ins=[num_d[:]], outs=[num_o[:]],
                        )
                        nc.sync.dma_start(nsum[:], num_o[:])
                    rden = spool.tile([1, B], F32, tag="rden")
                    nc.vector.reciprocal(rden[:], nsum[0:1, 4 * B:])
                    rbc = bpool.tile([128, B], F32, tag="bc")
                    nc.tensor.matmul(rbc[:], ones_sb[:], rden[:],
                                     start=True, stop=True)
                    Ps = nsum[:, :4 * B].rearrange("p (q b) -> p q b", b=B)
                    for q in range(4):
                        oq = spool.tile([128, B], F32, tag="oq%d" % (q % 2))
                        nc.vector.tensor_mul(oq[:], Ps[:, q], rbc[:])
                        nc.sync.dma_start(out[q * 128:(q + 1) * 128, :], oq[:])

    nc.finalize()
    return nc


# ======================= host side =======================================

def _freezer_x(w_ih, b_ih, b_hh):
    """x* with W_z x* + b_z ~= 26: z-gate saturates, h frozen at 0."""
    Wz = np.asarray(w_ih, np.float64)[Hd:2 * Hd]
    tgt = 26.0 - np.asarray(b_ih, np.float64)[Hd:2 * Hd] \
        - np.asarray(b_hh, np.float64)[Hd:2 * Hd]
    xs = Wz.T @ np.linalg.solve(Wz @ Wz.T, tgt)
    return xs.astype(np.float32)


def _fmt_x(xs, xd):
    """[B, TS, D] -> [DCH, 128, TS*B + PAD] in (t-major, b) order.
    Pad row D is the constant-1 bias row."""
    nb, Tn = xs.shape[0], xs.shape[1]
    xp = np.zeros((nb, Tn, DCH * 128), np.float32)
    xp[:, :, :D] = xs
    xp[:, :, D] = 1.0
    xt = xp.reshape(nb, Tn, DCH, 128).transpose(2, 3, 1, 0)
    flat = np.ascontiguousarray(xt).reshape(DCH, 128, Tn * nb)
    outa = np.zeros((DCH, 128, Tn * nb + PAD), np.float32)
    outa[:, :, :Tn * nb] = flat
    return outa.astype(xd)


def _prep_inputs(x, lengths, w_ih_f, w_hh_f, b_ih_f, b_hh_f,
                 w_ih_b, w_hh_b, b_ih_b, b_hh_b, att_w, att_b, ctx_v,
                 bf16=True):
    import ml_dtypes
    xd = ml_dtypes.bfloat16 if bf16 else np.float32
    f32 = np.float32
    x = np.asarray(x, f32)
    lengths = np.asarray(lengths).astype(np.int64)

    mask = (np.arange(T)[None, :] < lengths[:, None])
    xm = x * mask[:, :, None].astype(f32)
    xmr = np.ascontiguousarray(xm[:, ::-1])

    xf_f = _freezer_x(w_ih_f, b_ih_f, b_hh_f)
    xf_b = _freezer_x(w_ih_b, b_ih_b, b_hh_b)

    wb = np.zeros((128, NW), f32)
    for d, (wih, bi_, bh_) in enumerate(
            ((w_ih_f, b_ih_f, b_hh_f), (w_ih_b, b_ih_b, b_hh_b))):
        wt = np.asarray(wih, f32).T
        for c in range(DCH):
            d0, d1 = c * 128, min((c + 1) * 128, D)
            if d0 < D:
                wb[: d1 - d0, d * DCH * H3 + c * H3:
                   d * DCH * H3 + (c + 1) * H3] = wt[d0:d1]
        # bias rides the constant-1 pad row of x (chunk 2, row 44):
        # rz gates get b_ih+b_hh, n gates get b_ih only
        bias = np.asarray(bi_, f32).copy()
        bias[:2 * Hd] += np.asarray(bh_, f32)[:2 * Hd]
        wb[BIAS_ROW, d * DCH * H3 + 2 * H3: d * DCH * H3 + 3 * H3] = bias
    W_HH0 = 2 * DCH * H3
    for d, whh in enumerate((w_hh_f, w_hh_b)):
        ht = np.asarray(whh, f32).T
        for c in range(CCH):
            wb[:, W_HH0 + d * CCH * H3 + c * H3:
               W_HH0 + d * CCH * H3 + (c + 1) * H3] = ht[c * 128:(c + 1) * 128]
    AW0 = 2 * DCH * H3 + 2 * CCH * H3
    awt = np.asarray(att_w, f32).T
    for d in range(2):
        for c in range(CCH):
            q = d * 2 + c
            wb[:, AW0 + q * H: AW0 + (q + 1) * H] = awt[
                d * Hd + c * 128: d * Hd + (c + 1) * 128]
    CTX0 = AW0 + 4 * H
    wb[:, CTX0:CTX0 + 4] = np.asarray(ctx_v, f32).reshape(4, 128).T
    wb = wb.astype(xd)

    fb = np.zeros((128, NF), f32)
    for d, bh_ in enumerate((b_hh_f, b_hh_b)):
        bh = np.asarray(bh_, f32)
        for c in range(CCH):
            fb[:, d * CCH + c] = bh[2 * Hd + c * 128: 2 * Hd + (c + 1) * 128]
    fb[:, 2 * CCH:] = np.asarray(att_b, f32).reshape(4, 128).T

    tidx = np.arange(T)
    in_maps = []
    for c in range(NC):
        # fwd chain tokens: global tau = 128c - K + s
        t0 = c * ST - K
        xs_f = np.zeros((B, TS, D), f32)
        lo = max(0, -t0)
        xs_f[:, lo:] = xm[:, t0 + lo: t0 + TS]
        if lo:
            xs_f[:, :lo] = xf_f[None, None, :]
        # bwd chain: reversed-time tau_r = 128(7-c) - K + s
        t0r = (NC - 1 - c) * ST - K
        xs_b = np.zeros((B, TS, D), f32)
        lor = max(0, -t0r)
        xs_b[:, lor:] = xmr[:, t0r + lor: t0r + TS]
        if lor:
            xs_b[:, :lor] = xf_b[None, None, :]
        am = ((tidx[None, c * ST:(c + 1) * ST] < lengths[:, None])
              ).astype(f32)
        in_maps.append({
            "xTf": _fmt_x(xs_f, xd),
            "xTb": _fmt_x(xs_b, xd),
            "wblob": wb, "fblob": fb,
            "amask_d": np.ascontiguousarray(am.reshape(1, -1)),
        })
    return in_maps


_CACHED = {}

USE_BF16 = True


def kernel(**inputs):
    if "prog" not in _CACHED:
        _CACHED["prog"] = build_program(bf16=USE_BF16)
    nc = _CACHED["prog"]
    in_maps = _prep_inputs(**inputs, bf16=USE_BF16)
    res = run_bass_kernel_spmd(nc, in_maps, list(range(NC)))
    return np.ascontiguousarray(np.asarray(res.results[0]["out"], np.float32).T)


# revision 55
# speedup vs baseline: 1.2441x; 1.0861x over previous
"""Trainium2 Bass kernel for nn_AttentionHierarchy (BiGRU + attention pooling).

Time-sliced sharding (v2): wall time of the GRU recurrence is serial-steps x
per-step chain latency, so instead of data-parallel over batch we split TIME:

  core c (c=0..7): original-time slice t in [128c, 128c+128) for ALL 64 seqs,
  running TWO interleaved chains (fwd + bwd GRU, batch 64 each). Each chain
  starts K=32 steps early from h=0: the GRU forgets its initial state at
  ~0.5/step, so after 32 warmup steps the state matches the full scan to
  ~2e-6 (validated on the actual weight draw). Chains whose warmup would
  precede t=0 (core 0 fwd, core 7 bwd) get host-built "freezer" tokens:
  x* solves W_ih_z x* = 26 - b_z so the z-gate saturates to 1 and h stays
  exactly 0 through warmup.

  The bwd chain of core c covers reversed-time [128(7-c)-K, 128(8-c)), i.e.
  the SAME original-t range as its fwd slice, so attention (which mixes
  fwd/bwd halves at equal t) is core-local: no enc exchange at all. bwd enc
  is mirror-written so both tiles are t-ascending.

  Each core computes unnormalized attention partials for its slice
  (Num[h,b] = sum_t e^logit * enc, Den[b] = sum_t e^logit, masked); one
  small AllReduce(add) over all 8 cores combines them; out = Num/Den.

Per-step layouts match the v1 kernel: gates on partitions (6 chunks of 128),
batch on free dim; x pre-gates from a blocked 512-wide GEMM; h-recurrence as
12 weight-stationary matmuls (N=64) + 2 rank-1 bias matmuls per chain-step.
"""

import numpy as np
from contextlib import ExitStack

import concourse.bass as bass
import concourse.bacc as bacc
import concourse.mybir as mybir
from concourse import tile
from concourse.bass_utils import run_bass_kernel_spmd

F32 = mybir.dt.float32
AF = mybir.ActivationFunctionType
ALU = mybir.AluOpType
AX = mybir.AxisListType

B, T, D, H = 64, 1024, 300, 512
Hd = H // 2          # 256
H3 = 3 * Hd          # 768
NC = 8
ST = T // NC         # 128 slice steps per core
K = 16               # warmup steps (validated: h err 3.8e-3 at slice start,
                     # decays ~0.6x/step; negligible in pooled output)
TS = ST + K          # 160 chain steps per direction
SB = 16              # steps per block
NBLK = TS // SB      # 10
DCH = 3              # d chunks (300 -> 384)
CCH = 2              # hidden chunks (256)
MCH = 6              # gate chunks (768)
NTOK = TS * B        # tokens per chain
PAD = 2 * SB * B     # prefetch overrun pad (2 blocks)

NW = 2 * DCH * H3 + 2 * CCH * H3 + 4 * H + 4
NF = 2 * CCH + 4
BIAS_ROW = D - 2 * 128  # row 44 of the c=2 w_ih chunk carries the biases


def build_program(bf16=True, debug_stage=3):
    XD = mybir.dt.bfloat16 if bf16 else F32
    nc = bacc.Bacc()

    xT = [nc.dram_tensor(nm, [DCH, 128, NTOK + PAD], XD, kind="ExternalInput")
          for nm in ("xTf", "xTb")]
    wblob = nc.dram_tensor("wblob", [128, NW], XD, kind="ExternalInput")
    fblob = nc.dram_tensor("fblob", [128, NF], F32, kind="ExternalInput")
    amask_d = nc.dram_tensor("amask_d", [1, B * ST], F32, kind="ExternalInput")
    out = nc.dram_tensor("out", [H, B], F32, kind="ExternalOutput")
    enc = nc.dram_tensor("enc", [2, CCH, 128, B, TS], XD)
    num_d = nc.dram_tensor("num_d", [128, 4 * B + B], F32)
    num_o = nc.dram_tensor("num_o", [128, 4 * B + B], F32)

    # wblob column offsets
    W_IH = [0, DCH * H3]                       # per dir
    W_HH = [2 * DCH * H3, 2 * DCH * H3 + CCH * H3]
    AW0 = 2 * DCH * H3 + 2 * CCH * H3
    CTX0 = AW0 + 4 * H

    with ExitStack() as ctx:
        tc = ctx.enter_context(tile.TileContext(nc))
        wpool = ctx.enter_context(tc.tile_pool(name="weights", bufs=1))

        wblob_sb = wpool.tile([128, NW], XD)
        fblob_sb = wpool.tile([128, NF], F32)
        zcol = wpool.tile([128, 1], F32)
        ones_sb = wpool.tile([1, 128], F32)
        ones1 = wpool.tile([1, B], XD)
        nc.sync.dma_start(wblob_sb[:], wblob[:])
        nc.sync.dma_start(fblob_sb[:], fblob[:])
        ones2 = wpool.tile([128, CCH, B], F32)
        nc.gpsimd.memset(zcol[:], 0.0)
        nc.gpsimd.memset(ones_sb[:], 1.0)
        nc.gpsimd.memset(ones1[:], 1.0)
        nc.gpsimd.memset(ones2[:], 1.0)

        w_ih = [[wblob_sb[:, W_IH[d] + c * H3: W_IH[d] + (c + 1) * H3]
                 for c in range(DCH)] for d in range(2)]
        w_hh = [[wblob_sb[:, W_HH[d] + c * H3: W_HH[d] + (c + 1) * H3]
                 for c in range(CCH)] for d in range(2)]
        aw_sb = {(d, c): wblob_sb[:, AW0 + (d * 2 + c) * H: AW0 + (d * 2 + c + 1) * H]
                 for d in range(2) for c in range(CCH)}
        ctxv_sb = wblob_sb[:, CTX0:CTX0 + 4]
        bhn_col = [[fblob_sb[:, d * CCH + c: d * CCH + c + 1]
                    for c in range(CCH)] for d in range(2)]
        attb_sb = fblob_sb[:, 2 * CCH: 2 * CCH + 4]

        # ---------------- recurrence (both dirs interleaved) --------------
        with (
            tc.tile_pool(name="fx", bufs=2) as xpool,
            tc.tile_pool(name="fgx", bufs=1) as fgx,
            tc.tile_pool(name="rec_col", bufs=2) as rcol,
            tc.tile_pool(name="rec_tmp", bufs=2) as rtmp,
            tc.tile_pool(name="rec_ps", bufs=1, space="PSUM") as rps,
            tc.tile_pool(name="gps", bufs=2, space="PSUM") as gps,
            tc.tile_pool(name="rec_h", bufs=1) as rh,
            tc.tile_pool(name="enc_pool", bufs=2) as encp,
        ):
            h_tile = [rh.tile([128, CCH, B], XD, name="h%d" % d) for d in range(2)]
            for d in range(2):
                nc.vector.memset(h_tile[d][:], 0.0)
            # gx layout: [128, (ht, m, t8, b)] so each GEMM tile copy is a
            # contiguous [128, 512] DMA from PSUM
            gxbuf = [[fgx.tile([128, 2, MCH, 8, B], F32, name="gx%d%d" % (d, a))
                      for a in range(2)] for d in range(2)]

            def gemm_xdma(d, blk_i):
                xs = []
                for c in range(DCH):
                    xt = xpool.tile([128, SB * B], XD, tag="xs%d%d" % (d, c),
                                    name="xs%d%d" % (d, c))
                    nc.sync.dma_start(
                        xt[:], xT[d][c, :, bass.ds(blk_i * (SB * B), SB * B)])
                    xs.append(xt)
                return xs

            def gemm_piece(d, xs, out_gx, ht, m):
                ps = gps.tile([128, 512], F32, tag="gps")
                for c in range(DCH):
                    nc.tensor.matmul(
                        ps[:],
                        w_ih[d][c][:, m * 128:(m + 1) * 128],
                        xs[c][:, ht * 512:(ht + 1) * 512],
                        start=(c == 0),
                        stop=(c == DCH - 1),
                    )
                ov = out_gx[:, ht, m].rearrange("p t b -> p (t b)")
                if (ht * MCH + m) % 2 == 0:
                    nc.scalar.activation(ov, ps[:], AF.Copy)
                else:
                    nc.vector.tensor_copy(ov, ps[:])

            def gemm_half(d, xs, out_gx, ht):
                for m in range(MCH):
                    gemm_piece(d, xs, out_gx, ht, m)

            def gemm_block(d, blk_i, out_gx):
                xs = gemm_xdma(d, blk_i)
                for ht in range(2):
                    gemm_half(d, xs, out_gx, ht)

            h_last = [None, None]

            def chain_step_a(d, s, gx_cur, st):
                """Phase 1: h-matmuls, rzs, sigma, t2/pre, zbar/zh."""
                if s == 0:
                    h_prev = h_tile[d][:, :, :]
                else:
                    h_prev = h_last[d][:, :, :]
                gp_rz = rps.tile([128, 4, B], F32, tag="gprz%d" % d)
                gp_n = rps.tile([128, CCH, B], F32, tag="gpn%d" % d)
                for m in range(MCH):
                    tgt = gp_rz[:, m] if m < 4 else gp_n[:, m - 4]
                    for c in range(CCH):
                        nc.tensor.matmul(
                            tgt,
                            w_hh[d][c][:, m * 128:(m + 1) * 128],
                            h_prev[:, c],
                            start=(c == 0),
                            stop=(c == CCH - 1),
                        )
                gxv = gx_cur[:, s // 8, :, s % 8]
                rzs = rtmp.tile([128, 4, B], F32, tag="rzs%d" % d)
                nc.vector.tensor_add(rzs[:], gp_rz[:], gxv[:, :4])
                rz = rtmp.tile([128, 4, B], F32, tag="rz%d" % d)
                nc.scalar.activation(rz[:], rzs[:], AF.Sigmoid, bias=zcol[:])
                t2 = rtmp.tile([128, CCH, B], F32, tag="t2%d" % d)
                for c in range(CCH):
                    nc.vector.scalar_tensor_tensor(
                        t2[:, c], gp_n[:, c], bhn_col[d][c], rz[:, c],
                        ALU.add, ALU.mult)
                pre = rtmp.tile([128, CCH, B], F32, tag="pre%d" % d)
                nc.vector.tensor_add(pre[:], t2[:], gxv[:, 4:])
                zbar = rtmp.tile([128, CCH, B], F32, tag="zb%d" % d)
                nc.vector.scalar_tensor_tensor(
                    zbar[:], rz[:, CCH:], -1.0, ones2[:], ALU.mult, ALU.add)
                zh = rtmp.tile([128, CCH, B], F32, tag="zh%d" % d)
                nc.gpsimd.tensor_mul(zh[:], rz[:, CCH:], h_prev)
                st[d] = (pre, zbar, zh)

            def chain_step_b(d, s, cols, enc_t, st):
                """Phase 2: tanh, update, state copies."""
                pre, zbar, zh = st[d]
                n = rtmp.tile([128, CCH, B], F32, tag="n%d" % d)
                nc.scalar.activation(n[:], pre[:], AF.Tanh, bias=zcol[:])
                zbn = rtmp.tile([128, CCH, B], F32, tag="zn%d" % d)
                nc.vector.tensor_mul(zbn[:], zbar[:], n[:])
                cur = rtmp.tile([128, CCH, B], XD, tag="hc%d" % d,
                                name="hc%d" % d)
                nc.vector.tensor_add(cur[:], zbn[:], zh[:])
                h_last[d] = cur
                cslot = cols[s // 8][:, :, :, s % 8]
                if s % 2 == 0:
                    nc.scalar.activation(cslot, cur[:], AF.Copy)
                else:
                    nc.gpsimd.tensor_copy(cslot, cur[:])
                if s == SB - 1:
                    nc.vector.tensor_copy(h_tile[d][:], cur[:])
                if s % 8 == 7:
                    sub = s // 8
                    if d == 0:
                        dst = enc_t[:, :, :, bass.ds(sub * 8, 8)]
                    else:
                        dst = enc_t[:, :, :, bass.ds(8 - 8 * sub, 8)][:, :, :, ::-1]
                    nc.scalar.activation(dst, cols[sub][:], AF.Copy)

            def block(blk_i, ab, pf_ab):
                """One 16-step block for both dirs. blk_i: block index expr;
                ab: which gx buffer each dir consumes; pf_ab: prefetch target."""
                cols = [[rcol.tile([128, CCH, B, 8], XD,
                                   tag="col%d%d" % (d, kk),
                                   name="col%d%d" % (d, kk))
                         for kk in range(2)] for d in range(2)]
                enc_t = [encp.tile([128, CCH, B, SB], XD, tag="enc%d" % d,
                                   name="enct%d" % d) for d in range(2)]
                xs_pf = [None, None]
                st = [None, None]
                for s in range(SB):
                    for d in range(2):
                        chain_step_a(d, s, gxbuf[d][ab][:], st)
                        chain_step_b(d, s, cols[d], enc_t[d], st)
                    if s == 0:
                        xs_pf[0] = gemm_xdma(0, blk_i + 1)
                        xs_pf[1] = gemm_xdma(1, blk_i + 1)
                    elif s == 1:
                        gemm_half(0, xs_pf[0], gxbuf[0][pf_ab], 0)
                    elif s == 5:
                        gemm_half(0, xs_pf[0], gxbuf[0][pf_ab], 1)
                    elif s == 8:
                        gemm_half(1, xs_pf[1], gxbuf[1][pf_ab], 0)
                    elif s == 12:
                        gemm_half(1, xs_pf[1], gxbuf[1][pf_ab], 1)
                for d in range(2):
                    for c in range(CCH):
                        if d == 0:
                            dst = enc[0, c][:, :, bass.ds(blk_i * SB, SB)]
                        else:
                            dst = enc[1, c][:, :, bass.ds((NBLK - 1) * SB - blk_i * SB, SB)]
                        nc.sync.dma_start(dst, enc_t[d][:, c])

            gemm_block(0, 0, gxbuf[0][0])
            gemm_block(1, 0, gxbuf[1][0])
            with tc.For_i(0, NBLK // 2, 1) as i:
                block(i * 2, 0, 1)
                block(i * 2 + 1, 1, 0)
            if NBLK % 2:
                block(NBLK - 1, 0, 1)

        if debug_stage < 2:
            ov = out[:]
            for q in range(4):
                d, c = q // 2, q % 2
                with nc.allow_non_contiguous_dma(reason="debug"):
                    nc.gpsimd.dma_start(
                        ov[q * 128:(q + 1) * 128, :],
                        enc[d, c, :, :, K if d == 0 else 0],
                    )
        else:
            # ---------------- attention partials --------------------------
            with (
                tc.tile_pool(name="att_u", bufs=2) as upool,
                tc.tile_pool(name="att_s", bufs=2) as spool,
                tc.tile_pool(name="att_keep", bufs=1) as kpool,
            ):
                num_sb = kpool.tile([128, 4 * B + B], F32)
                nc.vector.memset(num_sb[:], 0.0)
                P = num_sb[:, :4 * B].rearrange("p (q b) -> p q b", b=B)
                dens = num_sb[0:1, 4 * B:]
                am_flat = kpool.tile([1, B * ST], F32)
                nc.sync.dma_start(am_flat[:], amask_d[:])
                # enc preload DMAs issued before the engine barrier so they
                # overlap the recurrence drain (DRAM deps order them after
                # the in-loop enc writes)
                bige = {}
                for d in range(2):
                    for c in range(CCH):
                        e = kpool.tile([128, B, ST], XD, name="be%d%d" % (d, c))
                        nc.sync.dma_start(
                            e[:], enc[d, c, :, :,
                                      bass.ds(K if d == 0 else 0, ST)])
                        bige[(d, c)] = e
                tc.strict_bb_all_engine_barrier()
                aps = ctx.enter_context(
                    tc.tile_pool(name="att_ps", bufs=1, space="PSUM"))
                lps = ctx.enter_context(
                    tc.tile_pool(name="att_lg", bufs=2, space="PSUM"))
                bpool = ctx.enter_context(
                    tc.tile_pool(name="att_bc", bufs=2, space="PSUM"))
                for j in range(B // 4):
                    b0 = 4 * j
                    encs = {k: v[:, b0:b0 + 4, :] for k, v in bige.items()}
                    ups = [aps.tile([128, 512], F32, tag="ups%d" % m,
                                    name="ups%d" % m) for m in range(4)]
                    ki = 0
                    for d in range(2):
                        for c in range(CCH):
                            rhs = encs[(d, c)][:].rearrange("p b t -> p (b t)")
                            for m in range(4):
                                nc.tensor.matmul(
                                    ups[m][:],
                                    aw_sb[(d, c)][:, m * 128:(m + 1) * 128],
                                    rhs,
                                    start=(ki == 0), stop=(ki == 3),
                                )
                            ki += 1
                    u_sb = upool.tile([128, 4, 512], XD, tag="usb")
                    for m in range(4):
                        nc.scalar.activation(
                            u_sb[:, m], ups[m][:], AF.Tanh,
                            bias=attb_sb[:, m:m + 1])
                    lg = lps.tile([1, 512], F32, tag="lg")
                    for m in range(4):
                        nc.tensor.matmul(
                            lg[:], ctxv_sb[:, m:m + 1], u_sb[:, m],
                            start=(m == 0), stop=(m == 3))
                    e_row = spool.tile([1, 512], F32, tag="erow")
                    nc.scalar.activation(e_row[:], lg[:], AF.Exp,
                                         bias=zcol[0:1, :])
                    if debug_stage == 25:
                        nc.sync.dma_start(out[b0 * 8:b0 * 8 + 8, :],
                                          e_row[:].rearrange("o (x b) -> (o x) b", b=B))
                        continue
                    em = spool.tile([1, 4 * ST], F32, tag="em")
                    nc.vector.tensor_mul(
                        em[:], e_row[:],
                        am_flat[0:1, b0 * ST:(b0 + 4) * ST])
                    nc.vector.tensor_reduce(
                        dens[0:1, b0:b0 + 4],
                        em[:].rearrange("o (b t) -> o b t", t=ST),
                        AX.X, ALU.add)
                    bc = bpool.tile([128, 4 * ST], F32, tag="bc")
                    nc.tensor.matmul(bc[:], ones_sb[:], em[:],
                                     start=True, stop=True)
                    bc_sb = spool.tile([128, 4 * ST], F32, tag="bcs")
                    nc.scalar.activation(bc_sb[:], bc[:], AF.Copy)
                    for d in range(2):
                        for c in range(CCH):
                            q = d * 2 + c
                            ef = encs[(d, c)][:].rearrange("p b t -> p (b t)")
                            scr = spool.tile([128, 4 * ST], F32,
                                             tag="scr%d" % (q % 2))
                            nc.vector.tensor_mul(scr[:], ef, bc_sb[:])
                            nc.vector.tensor_reduce(
                                P[:, q, b0:b0 + 4],
                                scr[:].rearrange("p (b t) -> p b t", t=ST),
                                AX.X, ALU.add)
                if debug_stage != 25:
                    nsum = kpool.tile([128, 4 * B + B], F32, name="nsum")
                    if debug_stage == 26:
                        nc.vector.tensor_copy(nsum[:], num_sb[:])
                    else:
                        nc.sync.dma_start(num_d[:], num_sb[:])
                        nc.gpsimd.collective_compute(
                            "AllReduce", ALU.add,
                            replica_groups=[list(range(NC))],
                            ins=[num_d[:]], outs=[num_o[:]],
                        )
                        nc.sync.dma_start(nsum[:], num_o[:])
                    rden = spool.tile([1, B], F32, tag="rden")
                    nc.vector.reciprocal(rden[:], nsum[0:1, 4 * B:])
                    rbc = bpool.tile([128, B], F32, tag="bc")
                    nc.tensor.matmul(rbc[:], ones_sb[:], rden[:],
                                     start=True, stop=True)
                    Ps = nsum[:, :4 * B].rearrange("p (q b) -> p q b", b=B)
                    for q in range(4):
                        oq = spool.tile([128, B], F32, tag="oq%d" % (q % 2))
                        nc.vector.tensor_mul(oq[:], Ps[:, q], rbc[:])
                        nc.sync.dma_start(out[q * 128:(q + 1) * 128, :], oq[:])

    nc.finalize()
    return nc


# ======================= host side =======================================

def _freezer_x(w_ih, b_ih, b_hh):
    """x* with W_z x* + b_z ~= 26: z-gate saturates, h frozen at 0."""
    Wz = np.asarray(w_ih, np.float64)[Hd:2 * Hd]
    tgt = 26.0 - np.asarray(b_ih, np.float64)[Hd:2 * Hd] \
        - np.asarray(b_hh, np.float64)[Hd:2 * Hd]
    xs = Wz.T @ np.linalg.solve(Wz @ Wz.T, tgt)
    return xs.astype(np.float32)


def _fmt_x(xs, xd):
    """[B, TS, D] -> [DCH, 128, TS*B + PAD] in (t-major, b) order.
    Pad row D is the constant-1 bias row."""
    nb, Tn = xs.shape[0], xs.shape[1]
    xp = np.zeros((nb, Tn, DCH * 128), np.float32)
    xp[:, :, :D] = xs
    xp[:, :, D] = 1.0
    xt = xp.reshape(nb, Tn, DCH, 128).transpose(2, 3, 1, 0)
    flat = np.ascontiguousarray(xt).reshape(DCH, 128, Tn * nb)
    outa = np.zeros((DCH, 128, Tn * nb + PAD), np.float32)
    outa[:, :, :Tn * nb] = flat
    return outa.astype(xd)


def _prep_inputs(x, lengths, w_ih_f, w_hh_f, b_ih_f, b_hh_f,
                 w_ih_b, w_hh_b, b_ih_b, b_hh_b, att_w, att_b, ctx_v,
                 bf16=True):
    import ml_dtypes
    xd = ml_dtypes.bfloat16 if bf16 else np.float32
    f32 = np.float32
    x = np.asarray(x, f32)
    lengths = np.asarray(lengths).astype(np.int64)

    mask = (np.arange(T)[None, :] < lengths[:, None])
    xm = x * mask[:, :, None].astype(f32)
    xmr = np.ascontiguousarray(xm[:, ::-1])

    xf_f = _freezer_x(w_ih_f, b_ih_f, b_hh_f)
    xf_b = _freezer_x(w_ih_b, b_ih_b, b_hh_b)

    wb = np.zeros((128, NW), f32)
    for d, (wih, bi_, bh_) in enumerate(
            ((w_ih_f, b_ih_f, b_hh_f), (w_ih_b, b_ih_b, b_hh_b))):
        wt = np.asarray(wih, f32).T
        for c in range(DCH):
            d0, d1 = c * 128, min((c + 1) * 128, D)
            if d0 < D:
                wb[: d1 - d0, d * DCH * H3 + c * H3:
                   d * DCH * H3 + (c + 1) * H3] = wt[d0:d1]
        # bias rides the constant-1 pad row of x (chunk 2, row 44):
        # rz gates get b_ih+b_hh, n gates get b_ih only
        bias = np.asarray(bi_, f32).copy()
        bias[:2 * Hd] += np.asarray(bh_, f32)[:2 * Hd]
        wb[BIAS_ROW, d * DCH * H3 + 2 * H3: d * DCH * H3 + 3 * H3] = bias
    W_HH0 = 2 * DCH * H3
    for d, whh in enumerate((w_hh_f, w_hh_b)):
        ht = np.asarray(whh, f32).T
        for c in range(CCH):
            wb[:, W_HH0 + d * CCH * H3 + c * H3:
               W_HH0 + d * CCH * H3 + (c + 1) * H3] = ht[c * 128:(c + 1) * 128]
    AW0 = 2 * DCH * H3 + 2 * CCH * H3
    awt = np.asarray(att_w, f32).T
    for d in range(2):
        for c in range(CCH):
            q = d * 2 + c
            wb[:, AW0 + q * H: AW0 + (q + 1) * H] = awt[
                d * Hd + c * 128: d * Hd + (c + 1) * 128]
    CTX0 = AW0 + 4 * H
    wb[:, CTX0:CTX0 + 4] = np.asarray(ctx_v, f32).reshape(4, 128).T
    wb = wb.astype(xd)

    fb = np.zeros((128, NF), f32)
    for d, bh_ in enumerate((b_hh_f, b_hh_b)):
        bh = np.asarray(bh_, f32)
        for c in range(CCH):
            fb[:, d * CCH + c] = bh[2 * Hd + c * 128: 2 * Hd + (c + 1) * 128]
    fb[:, 2 * CCH:] = np.asarray(att_b, f32).reshape(4, 128).T

    tidx = np.arange(T)
    in_maps = []
    for c in range(NC):
        # fwd chain tokens: global tau = 128c - K + s
        t0 = c * ST - K
        xs_f = np.zeros((B, TS, D), f32)
        lo = max(0, -t0)
        xs_f[:, lo:] = xm[:, t0 + lo: t0 + TS]
        if lo:
            xs_f[:, :lo] = xf_f[None, None, :]
        # bwd chain: reversed-time tau_r = 128(7-c) - K + s
        t0r = (NC - 1 - c) * ST - K
        xs_b = np.zeros((B, TS, D), f32)
        lor = max(0, -t0r)
        xs_b[:, lor:] = xmr[:, t0r + lor: t0r + TS]
        if lor:
            xs_b[:, :lor] = xf_b[None, None, :]
        am = ((tidx[None, c * ST:(c + 1) * ST] < lengths[:, None])
              ).astype(f32)
        in_maps.append({
            "xTf": _fmt_x(xs_f, xd),
            "xTb": _fmt_x(xs_b, xd),
            "wblob": wb, "fblob": fb,
            "amask_d": np.ascontiguousarray(am.reshape(1, -1)),
        })
    return in_maps


_CACHED = {}

USE_BF16 = True


def kernel(**inputs):
    if "prog" not in _CACHED:
        _CACHED["prog"] = build_program(bf16=USE_BF16)
    nc = _CACHED["prog"]
    in_maps = _prep_inputs(**inputs, bf16=USE_BF16)
    res = run_bass_kernel_spmd(nc, in_maps, list(range(NC)))
    return np.ascontiguousarray(np.asarray(res.results[0]["out"], np.float32).T)
